# revision 52
# baseline (speedup 1.0000x reference)
"""Trainium2 Bass kernel for nn_Attention (pooling attention).

Math (per batch b):
    u[b]     = W_score @ h_t[b]            (score = (hidden @ W_score) . h_t
                                            collapses to hidden . (W_score @ h_t))
    score[t] = hidden[b,t,:] . u[b]        (split: 11 t-tiles on DVE via fused
                                            scalar_tensor_tensor mul+reduce, 5
                                            t-tiles on the PE from transposed-
                                            staged chunks as stationary weights)
    p[t]     = exp(score[t] - 50)          (ScalarE, bf16, two fused-accum exps)
    ctx      = sum_t p[t] * hidden[b,t,:]  (PE bf16xf16 matmuls, UNNORMALIZED,
                                            pairs of t-tiles per matmul into one
                                            persistent [32, 512] PSUM tile via
                                            zero-padded p columns)
    out[b]   = tanh([ctx/s, h_t[b]] @ W_att)

Schedule notes (vs the 117us baseline):
  - the DVE mul+tree+reduce score chain (~5.3us/batch, an ~85us floor) is
    replaced by 11 fused scalar_tensor_tensor ops (345ns effective each,
    1x mode + DVE_READ_ACCUMULATOR) plus 5 t-tiles scored on the PE from
    host-staged transposed chunks (FWL stationary loads + N=1 matmuls),
    costing +25% HBM traffic.  This balances DVE (~3.8us/batch), DMA
    (1.36MB/batch at ~358GB/s = 3.8us), and PE (~3.3us/batch).
  - all setup constants ride ONE 274KB blob on the scalar HWDGE ring ahead
    of the flood (one DMA completion receipt); h_t^T and the identity are
    staged f16/transposed from the host so the u-chain is 2 f16 matmuls.
  - tile_set_cur_wait phases each batch in the Tile scheduler's virtual
    clock; without it the scheduler's optimistic DVE model reorders the ACT
    queue (ubc copies behind PE-dependent exps) and starves the DVE.
  - exp is split (DVE-scored columns don't wait on PE score matmuls); the
    u broadcast for batch b+2 is emitted between exp2 and the ctx matmuls.
  - ctx matmuls are paired: lhsT = [128, 2x32] zero-padded p blocks, rhs =
    [128, 512], all 128 of them accumulating into one persistent [64, 512]
    PSUM tile (row b = even-tile ctx of batch b, row 32+b = odd-tile ctx at
    columns 256:512, folded once at the end).  The p padding puts the odd
    block at partition 32 because operand partition bases must be 32-aligned.
  - p_pad zeroing is split DVE-memset/ACT-broadcast-copy so neither engine's
    queue is blocked during the head.
  - batches 0-1 get their u broadcast straight from the setup blob via a
    broadcast-lhsT matmul (htT column x W_score^T), skipping the
    ps_u -> u16-copy leg; moves the first DVE score op ~1us earlier and
    shifted the measured distribution from ~94-105us to ~93-96us.

Sharding: data-parallel over batch, 16 batches per core on 8 cores; weights
replicated.  hidden_states staged fp16 (plus a 5/16 transposed duplicate).
"""

import sys

import numpy as np

_TRN_REPO = "/opt/trn_rl_repo"
if _TRN_REPO not in sys.path:
    sys.path.insert(0, _TRN_REPO)

import concourse.bass as bass
import concourse.bacc as bacc
import concourse.tile as tile
from concourse import mybir
from concourse.bass_utils import run_bass_kernel_spmd

N_CORES = 8
B, T, H = 128, 2048, 256
NB = B // N_CORES  # batches per core
P = 128  # SBUF partitions
TT = T // P  # t-tiles per batch (16)
NDVE = 11  # t-tiles per batch scored on DVE (STT); the rest go to the PE
NPE = TT - NDVE  # t-tiles scored on PE from transposed-staged chunks
NPAIR = TT // 2  # paired ctx matmuls per batch
OUT_D = 128
YC = TT * H + 2 * NPE * P  # combined per-batch row: y block + yT chunks
EXP_SHIFT = -50.0  # keeps exp() in fp32/bf16 range; cancels in the softmax ratio

F32 = mybir.dt.float32
F16 = mybir.dt.float16
BF16 = mybir.dt.bfloat16


def _build_kernel(nc: bass.Bass, tc: "tile.TileContext", hiddenc, setupb, out):
    mult = mybir.AluOpType.mult

    from contextlib import ExitStack

    with ExitStack() as ctx:
        const = ctx.enter_context(tc.tile_pool(name="const", bufs=1))
        ybufs = ctx.enter_context(tc.tile_pool(name="ybufs", bufs=7))
        sc = ctx.enter_context(tc.tile_pool(name="sc", bufs=2))
        ubcp = ctx.enter_context(tc.tile_pool(name="ubcp", bufs=3))
        psum_t = ctx.enter_context(tc.tile_pool(name="psum_t", bufs=3, space="PSUM"))
        psum_u = ctx.enter_context(tc.tile_pool(name="psum_u", bufs=2, space="PSUM"))
        psum_p = ctx.enter_context(tc.tile_pool(name="psum_p", bufs=1, space="PSUM"))

        # ---- setup loads: ONE blob on the scalar HWDGE ring (one receipt),
        # ahead of any flood.  Layout: [htT(32) | wst(512) | watt(512) | id(16)]
        setup_sb = const.tile([P, 1072], F16, tag="setupb")
        nc.scalar.dma_start(out=setup_sb, in_=setupb[:, :])
        htT_sb = setup_sb[:, 0:32].rearrange("p (kk b) -> p kk b", kk=2)
        wst_sb = setup_sb[:, 32:544].rearrange("p (kk h) -> p kk h", kk=2)
        watt_sb = setup_sb[:, 544:1056].rearrange("p (dd j) -> p dd j", dd=4)
        ident16f = setup_sb[0:16, 1056:1072]

        # zero-padded p storage: per batch a [16i, 16col] strip; only col b is
        # ever written (by exp), so ctx matmuls see 0 for every other row of
        # the shared ctx accumulator.  Runs during the preamble/DMA warmup.
        p_pad = const.tile([P, NB, TT, 2 * NB], BF16, tag="p_pad")
        p_flat = p_pad.rearrange("p b i c -> p (b i c)")
        HALF_PAD = NB * TT * 2 * NB // 2
        nc.vector.memset(p_flat[:, 0:HALF_PAD], 0.0)

        ones_col = const.tile([P, 1], F32, tag="ones_col")
        nc.vector.memset(ones_col, 1.0)
        shift_col = const.tile([P, 1], F32, tag="shift_col")
        nc.vector.memset(shift_col, EXP_SHIFT)
        zero_col = const.tile([P, 1], BF16, tag="zero_col")
        nc.vector.memset(zero_col, 0.0)
        # second half of the p padding is zeroed on the idle ACT engine
        nc.scalar.copy(
            out=p_flat[:, HALF_PAD : 2 * HALF_PAD],
            in_=zero_col.broadcast_to([P, HALF_PAD]),
        )

        q_all = const.tile([P, 2, NB], F32, tag="q_all")

        # ---- y-load flood: 1.36MB per batch, alternating HWDGE rings -------
        ylist = {}

        def emit_load(b):
            eng = nc.sync if b % 2 == 0 else nc.scalar
            yc = ybufs.tile([P, YC], F16, tag="yc", name=f"yc_{b}")
            if b == 0:
                cut = (TT // 2) * H
                eng.dma_start(out=yc[:, 0:cut], in_=hiddenc[b][:, 0:cut])
                eng.dma_start(out=yc[:, cut:YC], in_=hiddenc[b][:, cut:YC])
            else:
                eng.dma_start(out=yc, in_=hiddenc[b])
            ylist[b] = yc

        PREF = 6  # batches of DMA-ahead
        for k in range(PREF):
            emit_load(k)

        # ---- u = h_t @ W_score^T (f16 matmuls, htT staged from host) -------
        ps_u = psum_t.tile([NB, H], F32, tag="ptmp")
        for half in range(2):
            nc.tensor.matmul(
                ps_u,
                lhsT=htT_sb[:, half, :],
                rhs=wst_sb[:, half, :],
                start=(half == 0),
                stop=(half == 1),
            )
        u16_sb = const.tile([NB, H], F16, tag="u16")
        nc.scalar.copy(out=u16_sb, in_=ps_u)
        # u^T halves [h-part, half, b] for the PE score tiles
        uT_sb = const.tile([P, 2, NB], F16, tag="uT")
        for half in range(2):
            ps_ut = psum_t.tile([P, NB], F32, tag="ptmp", name=f"ps_ut{half}")
            nc.tensor.matmul(
                ps_ut,
                lhsT=u16_sb[:, half * P : (half + 1) * P],
                rhs=ident16f,
                start=True,
                stop=True,
            )
            nc.scalar.copy(out=uT_sb[:, half, :], in_=ps_ut)

        # ---- persistent PSUM accumulators ----------------------------------
        # paired ctx for ALL batches: quadrants (b, h) and (16+b, 256+h) hold
        # the even-tile and odd-tile halves of batch b's unnormalized ctx.
        ctx_ps = psum_p.tile([4 * NB, 2 * H], F32, tag="ctx_all", name="ctx_all")
        # final output accumulator: do the h_t @ W_att half at setup time
        out_ps = psum_p.tile([NB, OUT_D], F32, tag="out_ps", name="out_ps")
        for dd in range(2, 4):
            nc.tensor.matmul(
                out_ps,
                lhsT=htT_sb[:, dd - 2, :],
                rhs=watt_sb[:, dd, :],
                start=(dd == 2),
                stop=False,
            )

        # u[b] broadcast to all partitions (PE ones-matmul + ACT copy to a
        # plain f16 tile for the STT in1); emitted 2 batches ahead of use.
        def emit_ubc(b):
            ubc_ps = psum_u.tile([P, H], F32, tag="ubc", name=f"ubc{b}")
            sel = ident16f[:, b : b + 1].broadcast_to([16, P])
            nc.tensor.matmul(ubc_ps, lhsT=sel, rhs=u16_sb, start=True, stop=True)
            return ubc_ps

        def emit_ubc_copy(b, ubc_ps):
            ubc16 = ubcp.tile([P, H], F16, tag="ubc16", name=f"ubc16_{b}")
            nc.scalar.copy(out=ubc16, in_=ubc_ps)
            return ubc16

        # batches 0-1: u broadcast computed directly from the setup blob
        # (broadcast-lhsT matmul), skipping the ps_u -> u16-copy leg so the
        # first DVE score op starts ~2.5us earlier.
        def emit_ubc_direct(b):
            ubc_ps = psum_u.tile([P, H], F32, tag="ubc", name=f"ubcd{b}")
            for half in range(2):
                nc.tensor.matmul(
                    ubc_ps,
                    lhsT=htT_sb[:, half, b : b + 1].broadcast_to([P, P]),
                    rhs=wst_sb[:, half, :],
                    start=(half == 0),
                    stop=(half == 1),
                )
            return ubc_ps

        pend = {}
        for b in range(2):
            pend[b] = emit_ubc_copy(b, emit_ubc_direct(b))

        # ---- per-batch pipeline --------------------------------------------
        # tile_wait_until phases each batch in the scheduler's virtual clock
        # so the ACT/PE queues can't be reordered across batches (the
        # scheduler's optimistic DVE model otherwise front-loads PE-dependent
        # exps and starves the DVE at runtime).  No runtime cost.
        PERIOD_MS = 4.2e-3
        for b in range(NB):
            tc.tile_set_cur_wait((b + 1) * PERIOD_MS)
            yc = ylist.pop(b)
            ubc16 = pend.pop(b)
            y16 = yc[:, 0 : TT * H].rearrange("p (i h) -> p i h", h=H)
            yt16 = yc[:, TT * H :].rearrange("p (c t) -> p c t", t=P)

            # score tiles [0, NDVE): fused DVE mul+reduce (fp32 accumulator)
            score32 = sc.tile([P, NDVE], F32, tag="score32", name=f"s32_{b}")
            for i in range(NDVE):
                z = sc.tile([P, H], F16, tag="z")
                nc.vector.scalar_tensor_tensor(
                    out=z,
                    in0=y16[:, i, :],
                    scalar=1.0,
                    in1=ubc16,
                    op0=mult,
                    op1=mult,
                    accum_out=score32[:, i : i + 1],
                )

            # score tiles [NDVE, TT): PE stationary-weight matmuls over the
            # transposed chunks (chunk^T @ u_half accumulated over h-halves
            # gives the [128t, 1] score column directly).
            scoreT_ps = psum_t.tile([P, NPE], F32, tag="ptmp", name=f"sT_{b}")
            for j in range(NPE):
                for half in range(2):
                    nc.tensor.matmul(
                        scoreT_ps[:, j : j + 1],
                        lhsT=yt16[:, 2 * j + half, :],
                        rhs=uT_sb[:, half, b : b + 1],
                        start=(half == 0),
                        stop=(half == 1),
                    )

            # p = exp(score - 50) in bf16 into the padded column strip; the
            # DVE-scored columns don't wait on the PE score matmuls.
            nc.scalar.activation(
                out=p_pad[:, b, 0:NDVE, b : b + 1].rearrange("p i c -> p (i c)"),
                in_=score32,
                func=mybir.ActivationFunctionType.Exp,
                bias=shift_col,
                scale=1.0,
                accum_out=q_all[:, 0, b : b + 1],
            )
            nc.scalar.activation(
                out=p_pad[:, b, NDVE:TT, b : b + 1].rearrange("p i c -> p (i c)"),
                in_=scoreT_ps,
                func=mybir.ActivationFunctionType.Exp,
                bias=shift_col,
                scale=1.0,
                accum_out=q_all[:, 1, b : b + 1],
            )

            # next-next batch's u broadcast rides the PE queue ahead of the
            # ctx matmuls so the DVE never starves on it.
            if b + 2 < NB:
                ubc_ps_next = emit_ubc(b + 2)

            # ctx, paired: quadrants (m<16, n<256) and (m>=16, n>=256) are
            # the even/odd tile halves; the other two quadrants are junk.
            for q in range(NPAIR):
                nc.tensor.matmul(
                    ctx_ps,
                    lhsT=p_pad[:, b, 2 * q : 2 * q + 2, :],
                    rhs=y16[:, 2 * q : 2 * q + 2, :],
                    start=(b == 0 and q == 0),
                    stop=(b == NB - 1 and q == NPAIR - 1),
                )

            if b + 2 < NB:
                pend[b + 2] = emit_ubc_copy(b + 2, ubc_ps_next)
            if b + PREF < NB:
                emit_load(b + PREF)

        # ---- finalize: s per batch, fold+normalize, transpose, @ W_att -----
        tc.tile_set_cur_wait((NB + 1) * PERIOD_MS)
        s_ps = psum_t.tile([NB, 1], F32, tag="ptmp", name="s_ps")
        for two in range(2):
            nc.tensor.matmul(
                s_ps, lhsT=q_all[:, two, :], rhs=ones_col,
                start=(two == 0), stop=(two == 1),
            )
        s_sb = sc.tile([NB, 1], F32, tag="s_sb")
        nc.scalar.copy(out=s_sb, in_=s_ps)
        rs_sb = sc.tile([NB, 1], F32, tag="rs_sb")
        nc.vector.reciprocal(out=rs_sb, in_=s_sb)

        ceven = sc.tile([NB, H], F32, tag="ceven")
        nc.scalar.copy(out=ceven, in_=ctx_ps[0:NB, 0:H])
        cfold = sc.tile([NB, H], F32, tag="cfold")
        nc.vector.tensor_add(cfold, ceven, ctx_ps[2 * NB : 3 * NB, H : 2 * H])
        preN = sc.tile([NB, H], F16, tag="preN")
        nc.vector.tensor_scalar_mul(preN, cfold, rs_sb)

        preT = sc.tile([P, 2, NB], F16, tag="preT")
        for j in range(2):
            pT_ps = psum_t.tile([P, NB], F32, tag="ptmp", name=f"pT{j}")
            nc.tensor.matmul(
                pT_ps,
                lhsT=preN[:, j * P : (j + 1) * P],
                rhs=ident16f,
                start=True,
                stop=True,
            )
            nc.scalar.copy(out=preT[:, j, :], in_=pT_ps)
        for dd in range(2):
            nc.tensor.matmul(
                out_ps,
                lhsT=preT[:, dd, :],
                rhs=watt_sb[:, dd, :],
                start=False,
                stop=(dd == 1),
            )
        out_sb = sc.tile([NB, OUT_D], F32, tag="out_sb")
        nc.scalar.activation(
            out=out_sb, in_=out_ps, func=mybir.ActivationFunctionType.Tanh
        )
        nc.sync.dma_start(out=out[:, :], in_=out_sb)


_NC_CACHE = {}


def _get_nc():
    if "nc" not in _NC_CACHE:
        nc = bacc.Bacc("TRN2", target_bir_lowering=False, debug=False)
        hiddenc = nc.declare_dram_parameter("hiddenc", [NB, P, YC], F16, isOutput=False)
        setupb = nc.declare_dram_parameter("setupb", [P, 1072], F16, isOutput=False)
        out = nc.declare_dram_parameter("out", [NB, OUT_D], F32, isOutput=True)
        with tile.TileContext(nc) as tc:
            _build_kernel(nc, tc, hiddenc, setupb, out)
        nc.compile()
        _NC_CACHE["nc"] = nc
    return _NC_CACHE["nc"]


def _stage(hidden_states, W_score, W_att):
    """Host-side staging shared by _run and tests."""
    hidden_states = np.asarray(hidden_states, dtype=np.float32)
    W_score = np.asarray(W_score, dtype=np.float32)
    W_att_s = np.ascontiguousarray(
        np.asarray(W_att, dtype=np.float16)
        .reshape(4, P, OUT_D).transpose(1, 0, 2).reshape(P, 4 * OUT_D)
    )
    nb_all = hidden_states.shape[0]
    hidden16 = hidden_states.astype(np.float16)
    # combined per-batch region: [t,h] tile block (t = p*TT + i) followed by
    # the transposed chunks for the PE-scored tiles (partition = h there).
    hv = hidden16.reshape(nb_all, P, TT, 2, P)  # [b, p, i, half, h]
    hiddenc = np.concatenate(
        [
            hidden16.reshape(nb_all, P, TT * H),
            hv[:, :, NDVE:, :, :].transpose(0, 4, 2, 3, 1).reshape(nb_all, P, 2 * NPE * P),
        ],
        axis=2,
    )
    wst = np.ascontiguousarray(
        W_score.T.astype(np.float16).reshape(2, P, H).transpose(1, 0, 2).reshape(P, 2 * H)
    )
    return hidden16, hiddenc, wst, W_att_s


def _setup_blob(hidden16_shard, wst, watt):
    # [P, 1072] f16: [htT(32) | wst(512) | watt(512) | ident(16)]
    htT = hidden16_shard[:, T - 1, :].T  # [256, NB]
    htT = htT.reshape(2, P, -1).transpose(1, 0, 2).reshape(P, -1)
    blob = np.zeros((P, 1072), dtype=np.float16)
    blob[:, 0:32] = htT
    blob[:, 32:544] = wst
    blob[:, 544:1056] = watt
    blob[0:16, 1056:1072] = np.eye(16, dtype=np.float16)
    return np.ascontiguousarray(blob)


def _run(hidden_states, W_score, W_att, trace=False, trace_kwargs=None):
    hidden16, hiddenc, wst, W_att_s = _stage(hidden_states, W_score, W_att)
    nc = _get_nc()
    in_maps = []
    for c in range(N_CORES):
        sl = slice(c * NB, (c + 1) * NB)
        in_maps.append(
            {
                "hiddenc": hiddenc[sl],
                "setupb": _setup_blob(hidden16[sl], wst, W_att_s),
            }
        )
    kwargs = {}
    if trace:
        kwargs["trace"] = True
        if trace_kwargs:
            kwargs.update(trace_kwargs)
    res = run_bass_kernel_spmd(nc, in_maps, list(range(N_CORES)), **kwargs)
    out = np.concatenate([res.results[c]["out"] for c in range(N_CORES)], axis=0)
    return out, res


def kernel(hidden_states, W_score, W_att):
    out, _ = _run(hidden_states, W_score, W_att, trace=False)
    return out


# revision 53
# speedup vs baseline: 1.1461x; 1.1461x over previous
"""Trainium2 Bass kernel for nn_Attention (pooling attention).

Math (per batch b):
    u[b]     = W_score @ h_t[b]            (score = (hidden @ W_score) . h_t
                                            collapses to hidden . (W_score @ h_t))
    score[t] = hidden[b,t,:] . u[b]        (split: 11 t-tiles on DVE via fused
                                            scalar_tensor_tensor mul+reduce, 5
                                            t-tiles on the PE from transposed-
                                            staged chunks as stationary weights)
    p[t]     = exp(score[t] - 50)          (ScalarE, bf16, two fused-accum exps)
    ctx      = sum_t p[t] * hidden[b,t,:]  (PE bf16xf16 matmuls, UNNORMALIZED,
                                            pairs of t-tiles per matmul into one
                                            persistent [32, 512] PSUM tile via
                                            zero-padded p columns)
    out[b]   = tanh([ctx/s, h_t[b]] @ W_att)

Schedule notes (vs the 117us baseline):
  - the DVE mul+tree+reduce score chain (~5.3us/batch, an ~85us floor) is
    replaced by 11 fused scalar_tensor_tensor ops (345ns effective each,
    1x mode + DVE_READ_ACCUMULATOR) plus 5 t-tiles scored on the PE from
    host-staged transposed chunks (FWL stationary loads + N=1 matmuls),
    costing +25% HBM traffic.  This balances DVE (~3.8us/batch), DMA
    (1.36MB/batch at ~358GB/s = 3.8us), and PE (~3.3us/batch).
  - all setup constants ride ONE 274KB blob on the scalar HWDGE ring ahead
    of the flood (one DMA completion receipt); h_t^T and the identity are
    staged f16/transposed from the host so the u-chain is 2 f16 matmuls.
  - tile_set_cur_wait phases each batch in the Tile scheduler's virtual
    clock; without it the scheduler's optimistic DVE model reorders the ACT
    queue (ubc copies behind PE-dependent exps) and starves the DVE.
  - exp is split (DVE-scored columns don't wait on PE score matmuls); the
    u broadcast for batch b+2 is emitted between exp2 and the ctx matmuls.
  - ctx matmuls are paired: lhsT = [128, 2x32] zero-padded p blocks, rhs =
    [128, 512], all 128 of them accumulating into one persistent [64, 512]
    PSUM tile (row b = even-tile ctx of batch b, row 32+b = odd-tile ctx at
    columns 256:512, folded once at the end).  The p padding puts the odd
    block at partition 32 because operand partition bases must be 32-aligned.
  - p_pad zeroing is split DVE-memset/ACT-broadcast-copy so neither engine's
    queue is blocked during the head.
  - batches 0-1 get their u broadcast straight from the setup blob via a
    broadcast-lhsT matmul (htT column x W_score^T), skipping the
    ps_u -> u16-copy leg; moves the first DVE score op ~1us earlier and
    shifted the measured distribution from ~94-105us to ~93-96us.

Sharding: data-parallel over batch, 16 batches per core on 8 cores; weights
replicated.  hidden_states staged fp16 (plus a 5/16 transposed duplicate).
"""

import sys

import numpy as np

_TRN_REPO = "/opt/trn_rl_repo"
if _TRN_REPO not in sys.path:
    sys.path.insert(0, _TRN_REPO)

import concourse.bass as bass
import concourse.bacc as bacc
import concourse.tile as tile
from concourse import mybir
from concourse.bass_utils import run_bass_kernel_spmd

N_CORES = 8
B, T, H = 128, 2048, 256
NB = B // N_CORES  # batches per core
P = 128  # SBUF partitions
TT = T // P  # t-tiles per batch (16)
NDVE = 11  # t-tiles per batch scored on DVE (STT); the rest go to the PE
NPE = TT - NDVE  # t-tiles scored on PE from transposed-staged chunks
NPAIR = TT // 2  # paired ctx matmuls per batch
OUT_D = 128
YC = TT * H + 2 * NPE * P  # combined per-batch row: y block + yT chunks
EXP_SHIFT = -50.0  # keeps exp() in fp32/bf16 range; cancels in the softmax ratio

F32 = mybir.dt.float32
F16 = mybir.dt.float16
BF16 = mybir.dt.bfloat16


def _build_kernel(nc: bass.Bass, tc: "tile.TileContext", hiddenc, setupb, out):
    mult = mybir.AluOpType.mult

    from contextlib import ExitStack

    with ExitStack() as ctx:
        const = ctx.enter_context(tc.tile_pool(name="const", bufs=1))
        ybufs = ctx.enter_context(tc.tile_pool(name="ybufs", bufs=6))
        sc = ctx.enter_context(tc.tile_pool(name="sc", bufs=2))
        ubcp = ctx.enter_context(tc.tile_pool(name="ubcp", bufs=3))
        psum_t = ctx.enter_context(tc.tile_pool(name="psum_t", bufs=3, space="PSUM"))
        psum_u = ctx.enter_context(tc.tile_pool(name="psum_u", bufs=2, space="PSUM"))
        psum_p = ctx.enter_context(tc.tile_pool(name="psum_p", bufs=1, space="PSUM"))

        # ---- setup loads: ONE blob on the scalar HWDGE ring (one receipt),
        # ahead of any flood.  Layout: [htT(32) | wst(512) | watt(512) | id(16)]
        setup_sb = const.tile([P, 1072], F16, tag="setupb")
        nc.scalar.dma_start(out=setup_sb, in_=setupb[:, :])
        htT_sb = setup_sb[:, 0:32].rearrange("p (kk b) -> p kk b", kk=2)
        wst_sb = setup_sb[:, 32:544].rearrange("p (kk h) -> p kk h", kk=2)
        watt_sb = setup_sb[:, 544:1056].rearrange("p (dd j) -> p dd j", dd=4)
        ident16f = setup_sb[0:16, 1056:1072]

        # zero-padded p storage: per batch a [16i, 16col] strip; only col b is
        # ever written (by exp), so ctx matmuls see 0 for every other row of
        # the shared ctx accumulator.  Runs during the preamble/DMA warmup.
        p_pad = const.tile([P, NB, TT, 2 * NB], BF16, tag="p_pad")
        p_flat = p_pad.rearrange("p b i c -> p (b i c)")
        HALF_PAD = NB * TT * 2 * NB // 2
        nc.vector.memset(p_flat[:, 0:HALF_PAD], 0.0)

        ones_col = const.tile([P, 1], F32, tag="ones_col")
        nc.vector.memset(ones_col, 1.0)
        shift_col = const.tile([P, 1], F32, tag="shift_col")
        nc.vector.memset(shift_col, EXP_SHIFT)
        zero_col = const.tile([P, 1], BF16, tag="zero_col")
        nc.vector.memset(zero_col, 0.0)
        # second half of the p padding is zeroed on the idle ACT engine
        nc.scalar.copy(
            out=p_flat[:, HALF_PAD : 2 * HALF_PAD],
            in_=zero_col.broadcast_to([P, HALF_PAD]),
        )

        q_all = const.tile([P, 2, NB], F32, tag="q_all")

        # ---- y-load flood: 1.36MB per batch, alternating HWDGE rings -------
        ylist = {}

        def emit_load(b):
            eng = nc.sync if b % 2 == 0 else nc.scalar
            yc = ybufs.tile([P, YC], F16, tag="yc", name=f"yc_{b}")
            if b == 0:
                cut = (TT // 2) * H
                eng.dma_start(out=yc[:, 0:cut], in_=hiddenc[b][:, 0:cut])
                eng.dma_start(out=yc[:, cut:YC], in_=hiddenc[b][:, cut:YC])
            else:
                eng.dma_start(out=yc, in_=hiddenc[b])
            ylist[b] = yc

        PREF = 5  # batches of DMA-ahead
        for k in range(PREF):
            emit_load(k)

        # ---- u = h_t @ W_score^T (f16 matmuls, htT staged from host) -------
        ps_u = psum_t.tile([NB, H], F32, tag="ptmp")
        for half in range(2):
            nc.tensor.matmul(
                ps_u,
                lhsT=htT_sb[:, half, :],
                rhs=wst_sb[:, half, :],
                start=(half == 0),
                stop=(half == 1),
            )
        u16_sb = const.tile([NB, H], F16, tag="u16")
        nc.scalar.copy(out=u16_sb, in_=ps_u)
        # u^T halves [h-part, half, b] for the PE score tiles
        uT_sb = const.tile([P, 2, NB], F16, tag="uT")
        for half in range(2):
            ps_ut = psum_t.tile([P, NB], F32, tag="ptmp", name=f"ps_ut{half}")
            nc.tensor.matmul(
                ps_ut,
                lhsT=u16_sb[:, half * P : (half + 1) * P],
                rhs=ident16f,
                start=True,
                stop=True,
            )
            nc.scalar.copy(out=uT_sb[:, half, :], in_=ps_ut)

        # ---- persistent PSUM accumulators ----------------------------------
        # paired ctx for ALL batches: quadrants (b, h) and (16+b, 256+h) hold
        # the even-tile and odd-tile halves of batch b's unnormalized ctx.
        ctx_ps = psum_p.tile([4 * NB, 2 * H], F32, tag="ctx_all", name="ctx_all")
        # final output accumulator: do the h_t @ W_att half at setup time
        out_ps = psum_p.tile([NB, OUT_D], F32, tag="out_ps", name="out_ps")
        for dd in range(2, 4):
            nc.tensor.matmul(
                out_ps,
                lhsT=htT_sb[:, dd - 2, :],
                rhs=watt_sb[:, dd, :],
                start=(dd == 2),
                stop=False,
            )

        # u[b] broadcast to all partitions (PE ones-matmul + ACT copy to a
        # plain f16 tile for the STT in1); emitted 2 batches ahead of use.
        def emit_ubc(b):
            ubc_ps = psum_u.tile([P, H], F32, tag="ubc", name=f"ubc{b}")
            sel = ident16f[:, b : b + 1].broadcast_to([16, P])
            nc.tensor.matmul(ubc_ps, lhsT=sel, rhs=u16_sb, start=True, stop=True)
            return ubc_ps

        def emit_ubc_copy(b, ubc_ps):
            ubc16 = ubcp.tile([P, H], F16, tag="ubc16", name=f"ubc16_{b}")
            nc.scalar.copy(out=ubc16, in_=ubc_ps)
            return ubc16

        # batches 0-1: u broadcast computed directly from the setup blob
        # (broadcast-lhsT matmul), skipping the ps_u -> u16-copy leg so the
        # first DVE score op starts ~2.5us earlier.
        def emit_ubc_direct(b):
            ubc_ps = psum_u.tile([P, H], F32, tag="ubc", name=f"ubcd{b}")
            for half in range(2):
                nc.tensor.matmul(
                    ubc_ps,
                    lhsT=htT_sb[:, half, b : b + 1].broadcast_to([P, P]),
                    rhs=wst_sb[:, half, :],
                    start=(half == 0),
                    stop=(half == 1),
                )
            return ubc_ps

        pend = {}
        for b in range(2):
            pend[b] = emit_ubc_copy(b, emit_ubc_direct(b))

        # ---- per-batch pipeline --------------------------------------------
        # tile_wait_until phases each batch in the scheduler's virtual clock
        # so the ACT/PE queues can't be reordered across batches (the
        # scheduler's optimistic DVE model otherwise front-loads PE-dependent
        # exps and starves the DVE at runtime).  No runtime cost.
        PERIOD_MS = 4.2e-3
        for b in range(NB):
            tc.tile_set_cur_wait((b + 1) * PERIOD_MS)
            yc = ylist.pop(b)
            ubc16 = pend.pop(b)
            y16 = yc[:, 0 : TT * H].rearrange("p (i h) -> p i h", h=H)
            yt16 = yc[:, TT * H :].rearrange("p (c t) -> p c t", t=P)

            # score tiles [0, NDVE): fused DVE mul+reduce (fp32 accumulator)
            score32 = sc.tile([P, NDVE], F32, tag="score32", name=f"s32_{b}")
            for i in range(NDVE):
                z = sc.tile([P, H], F16, tag="z")
                nc.vector.scalar_tensor_tensor(
                    out=z,
                    in0=y16[:, i, :],
                    scalar=1.0,
                    in1=ubc16,
                    op0=mult,
                    op1=mult,
                    accum_out=score32[:, i : i + 1],
                )

            # score tiles [NDVE, TT): PE stationary-weight matmuls over the
            # transposed chunks (chunk^T @ u_half accumulated over h-halves
            # gives the [128t, 1] score column directly).
            scoreT_ps = psum_t.tile([P, NPE], F32, tag="ptmp", name=f"sT_{b}")
            for j in range(NPE):
                for half in range(2):
                    nc.tensor.matmul(
                        scoreT_ps[:, j : j + 1],
                        lhsT=yt16[:, 2 * j + half, :],
                        rhs=uT_sb[:, half, b : b + 1],
                        start=(half == 0),
                        stop=(half == 1),
                    )

            # p = exp(score - 50) in bf16 into the padded column strip; the
            # DVE-scored columns don't wait on the PE score matmuls.
            nc.scalar.activation(
                out=p_pad[:, b, 0:NDVE, b : b + 1].rearrange("p i c -> p (i c)"),
                in_=score32,
                func=mybir.ActivationFunctionType.Exp,
                bias=shift_col,
                scale=1.0,
                accum_out=q_all[:, 0, b : b + 1],
            )
            nc.scalar.activation(
                out=p_pad[:, b, NDVE:TT, b : b + 1].rearrange("p i c -> p (i c)"),
                in_=scoreT_ps,
                func=mybir.ActivationFunctionType.Exp,
                bias=shift_col,
                scale=1.0,
                accum_out=q_all[:, 1, b : b + 1],
            )

            # next-next batch's u broadcast rides the PE queue ahead of the
            # ctx matmuls so the DVE never starves on it.
            if b + 2 < NB:
                ubc_ps_next = emit_ubc(b + 2)

            # ctx, paired: quadrants (m<16, n<256) and (m>=16, n>=256) are
            # the even/odd tile halves; the other two quadrants are junk.
            for q in range(NPAIR):
                nc.tensor.matmul(
                    ctx_ps,
                    lhsT=p_pad[:, b, 2 * q : 2 * q + 2, :],
                    rhs=y16[:, 2 * q : 2 * q + 2, :],
                    start=(b == 0 and q == 0),
                    stop=(b == NB - 1 and q == NPAIR - 1),
                )

            if b + 2 < NB:
                pend[b + 2] = emit_ubc_copy(b + 2, ubc_ps_next)
            if b + PREF < NB:
                emit_load(b + PREF)

        # ---- finalize: s per batch, fold+normalize, transpose, @ W_att -----
        tc.tile_set_cur_wait((NB + 1) * PERIOD_MS)
        s_ps = psum_t.tile([NB, 1], F32, tag="ptmp", name="s_ps")
        for two in range(2):
            nc.tensor.matmul(
                s_ps, lhsT=q_all[:, two, :], rhs=ones_col,
                start=(two == 0), stop=(two == 1),
            )
        s_sb = sc.tile([NB, 1], F32, tag="s_sb")
        nc.scalar.copy(out=s_sb, in_=s_ps)
        rs_sb = sc.tile([NB, 1], F32, tag="rs_sb")
        nc.vector.reciprocal(out=rs_sb, in_=s_sb)

        ceven = sc.tile([NB, H], F32, tag="ceven")
        nc.scalar.copy(out=ceven, in_=ctx_ps[0:NB, 0:H])
        cfold = sc.tile([NB, H], F32, tag="cfold")
        nc.vector.tensor_add(cfold, ceven, ctx_ps[2 * NB : 3 * NB, H : 2 * H])
        preN = sc.tile([NB, H], F16, tag="preN")
        nc.vector.tensor_scalar_mul(preN, cfold, rs_sb)

        preT = sc.tile([P, 2, NB], F16, tag="preT")
        for j in range(2):
            pT_ps = psum_t.tile([P, NB], F32, tag="ptmp", name=f"pT{j}")
            nc.tensor.matmul(
                pT_ps,
                lhsT=preN[:, j * P : (j + 1) * P],
                rhs=ident16f,
                start=True,
                stop=True,
            )
            nc.scalar.copy(out=preT[:, j, :], in_=pT_ps)
        for dd in range(2):
            nc.tensor.matmul(
                out_ps,
                lhsT=preT[:, dd, :],
                rhs=watt_sb[:, dd, :],
                start=False,
                stop=(dd == 1),
            )
        out_sb = sc.tile([NB, OUT_D], F32, tag="out_sb")
        nc.scalar.activation(
            out=out_sb, in_=out_ps, func=mybir.ActivationFunctionType.Tanh
        )
        nc.sync.dma_start(out=out[:, :], in_=out_sb)


_NC_CACHE = {}


def _get_nc():
    if "nc" not in _NC_CACHE:
        nc = bacc.Bacc("TRN2", target_bir_lowering=False, debug=False)
        hiddenc = nc.declare_dram_parameter("hiddenc", [NB, P, YC], F16, isOutput=False)
        setupb = nc.declare_dram_parameter("setupb", [P, 1072], F16, isOutput=False)
        out = nc.declare_dram_parameter("out", [NB, OUT_D], F32, isOutput=True)
        with tile.TileContext(nc) as tc:
            _build_kernel(nc, tc, hiddenc, setupb, out)
        nc.compile()
        _NC_CACHE["nc"] = nc
    return _NC_CACHE["nc"]


def _stage(hidden_states, W_score, W_att):
    """Host-side staging shared by _run and tests."""
    hidden_states = np.asarray(hidden_states, dtype=np.float32)
    W_score = np.asarray(W_score, dtype=np.float32)
    W_att_s = np.ascontiguousarray(
        np.asarray(W_att, dtype=np.float16)
        .reshape(4, P, OUT_D).transpose(1, 0, 2).reshape(P, 4 * OUT_D)
    )
    nb_all = hidden_states.shape[0]
    hidden16 = hidden_states.astype(np.float16)
    # combined per-batch region: [t,h] tile block (t = p*TT + i) followed by
    # the transposed chunks for the PE-scored tiles (partition = h there).
    hv = hidden16.reshape(nb_all, P, TT, 2, P)  # [b, p, i, half, h]
    hiddenc = np.concatenate(
        [
            hidden16.reshape(nb_all, P, TT * H),
            hv[:, :, NDVE:, :, :].transpose(0, 4, 2, 3, 1).reshape(nb_all, P, 2 * NPE * P),
        ],
        axis=2,
    )
    wst = np.ascontiguousarray(
        W_score.T.astype(np.float16).reshape(2, P, H).transpose(1, 0, 2).reshape(P, 2 * H)
    )
    return hidden16, hiddenc, wst, W_att_s


def _setup_blob(hidden16_shard, wst, watt):
    # [P, 1072] f16: [htT(32) | wst(512) | watt(512) | ident(16)]
    htT = hidden16_shard[:, T - 1, :].T  # [256, NB]
    htT = htT.reshape(2, P, -1).transpose(1, 0, 2).reshape(P, -1)
    blob = np.zeros((P, 1072), dtype=np.float16)
    blob[:, 0:32] = htT
    blob[:, 32:544] = wst
    blob[:, 544:1056] = watt
    blob[0:16, 1056:1072] = np.eye(16, dtype=np.float16)
    return np.ascontiguousarray(blob)


def _run(hidden_states, W_score, W_att, trace=False, trace_kwargs=None):
    hidden16, hiddenc, wst, W_att_s = _stage(hidden_states, W_score, W_att)
    nc = _get_nc()
    in_maps = []
    for c in range(N_CORES):
        sl = slice(c * NB, (c + 1) * NB)
        in_maps.append(
            {
                "hiddenc": hiddenc[sl],
                "setupb": _setup_blob(hidden16[sl], wst, W_att_s),
            }
        )
    kwargs = {}
    if trace:
        kwargs["trace"] = True
        if trace_kwargs:
            kwargs.update(trace_kwargs)
    res = run_bass_kernel_spmd(nc, in_maps, list(range(N_CORES)), **kwargs)
    out = np.concatenate([res.results[c]["out"] for c in range(N_CORES)], axis=0)
    return out, res


def kernel(hidden_states, W_score, W_att):
    out, _ = _run(hidden_states, W_score, W_att, trace=False)
    return out
